# revision 38
# baseline (speedup 1.0000x reference)
"""Distributed 2-layer GCN (BangaloreGCN) on 8 Trainium2 NeuronCores.

Matmul-aggregation design (v3):
  * Source-partitioned: core c owns nodes [c*6250, (c+1)*6250) and the
    edges whose SOURCE it owns.  Per layer, each core computes a local
    message table (dinv-scaled dense transform of its own nodes), then
    gathers per-edge messages with dma_gather in DEST-SORTED order.
  * The scatter side is done on the PE array instead of dma_scatter_add:
    the global dest space is split into 400 windows of 128 slots.  Each
    128-edge gathered tile (edge i -> partition i%128) is multiplied by
    a one-hot "selection" matrix S [128 edges, 128 dests] built on the
    DVE (is_equal of per-edge dest-column vs an iota row), accumulating
    partial sums for a window directly in PSUM.  This removes the
    scatter DMA, the accumulator zeroing, and the scatter descriptor
    generation of v2 entirely.
  * Node -> slot assignment is chosen by a greedy packer so that every
    window needs at most 2 tiles per source core (max in-window edge
    count <= 256 for all 8 cores): the SPMD-static stream is ~103k
    indices per core vs 800k/8 = 100k real edges.
  * Slot labeling ell = p*50 + w makes the per-core accumulator chunk
    contiguous per partition, so the PSUM->DRAM staging writes run at
    full DMA rate, and the ReduceScatter chunk c is exactly core c's
    own slots.  Both layers share the identical edge stream, gather
    indices, and S structure (S is rebuilt per layer; it does not fit
    in SBUF).
"""

import sys

sys.path.insert(0, "/opt/trn_rl_repo")

import ml_dtypes
import numpy as np

F16 = np.float16

# ---- problem constants ----
N_NODES = 50000
IN_CH = 128
HID = 64
HID2 = 32
BN_EPS = 1e-5

NCORES = 8
P = 128
WPC = 50                   # windows (tiles) per core
SPC = P * WPC              # 6400 slots per core
NSLOT = NCORES * SPC       # 51200
NWIN = NCORES * WPC        # 400 global windows
REAL = N_NODES // NCORES   # 6250 real nodes per core
WCAP = 127                 # real nodes per window (p=127 spare everywhere)
TBW = 128                  # table row width in bf16 elems (256B stride)
SPARE_ROW = WCAP * WPC     # a slot that is spare on every core (p=127,w=0)
NTBIN = int(__import__("os").environ.get("KNTBIN", "48"))  # tiles per gather bin
# ReduceScatter segments (windows per core): each segment's RS fires as
# soon as its windows are staged, overlapping the rest of the stream; the
# LAST (smallest) segment is the only RS latency left exposed.
SEGW = [20, 20, 10]
SEGOFF = [0, 20, 40]
NSEG = len(SEGW)
SEGSTARTW = [0]            # first global window of each segment
for _s in range(1, NSEG):
    SEGSTARTW.append(SEGSTARTW[-1] + NCORES * SEGW[_s - 1])
SEGROW = [NSLOT * o // WPC for o in SEGOFF]   # first acc row of each segment


# ----------------------------------------------------------------------
# host-side preparation
# ----------------------------------------------------------------------
def _wrap_idx(arr):
    """[n] int -> [128, n/16] int16 image (16-partition wrap, replicated)."""
    ni = arr.shape[0]
    assert ni % 16 == 0
    blk = arr.reshape(ni // 16, 16).T.astype(np.int16)
    return np.tile(blk, (8, 1))


def _pack_windows(Mi):
    """Greedy: assign nodes (rows of Mi [REAL, 8] = per-source-core indeg)
    to WPC windows, minimizing the max per-core in-window load, capped at
    WCAP nodes per window.  Returns win[i] for nodes in degree-sorted
    order and that order."""
    srt = np.argsort(-Mi.sum(1), kind="stable")
    Ms = Mi[srt]
    loads = np.zeros((WPC, NCORES), np.int64)
    cnt = np.zeros(WPC, np.int64)
    win = np.empty(REAL, np.int64)
    big = 1 << 40
    for i in range(REAL):
        cand = (loads + Ms[i]).max(1) + (cnt >= WCAP) * big
        w = int(np.argmin(cand))
        win[i] = w
        loads[w] += Ms[i]
        cnt[w] += 1
    return srt, win


def host_prep(x, edge_index, W1, b1, W2, b2, fcW, fcb,
              g1, be1, rm1, rv1, g2, be2, rm2, rv2):
    row = np.asarray(edge_index[0], np.int64)
    col = np.asarray(edge_index[1], np.int64)
    x = np.asarray(x, np.float32)

    deg = np.bincount(col, minlength=N_NODES).astype(np.float32) + 1.0
    dinv = (1.0 / np.sqrt(deg)).astype(np.float32)

    owner_src = row // REAL

    # ---- node -> slot assignment (window packing per dest core) ----
    M = np.zeros((N_NODES, NCORES), np.int32)
    np.add.at(M, (col, owner_src), 1)
    slot_of_node = np.full(N_NODES, -1, np.int64)      # global slot
    node_of_slot = np.full((NCORES, SPC), -1, np.int64)
    for c in range(NCORES):
        nodes = np.arange(c * REAL, (c + 1) * REAL)
        srt, win = _pack_windows(M[nodes])
        # p = rank within window (stable in assignment order)
        o2 = np.argsort(win, kind="stable")
        wsort = win[o2]
        first = np.zeros(REAL, np.int64)
        starts = np.r_[0, np.flatnonzero(np.diff(wsort)) + 1]
        first[starts] = starts
        first = np.maximum.accumulate(first)
        p_of = np.empty(REAL, np.int64)
        p_of[o2] = np.arange(REAL) - first
        assert p_of.max() < WCAP
        ell = p_of * WPC + win
        slot_of_node[nodes[srt]] = c * SPC + ell
        node_of_slot[c, ell] = nodes[srt]

    # ---- per-edge window/column/source-row ----
    # Global window order is SEGMENT-major, then dest core, then window:
    # segment s's ReduceScatter covers acc rows [SEGROW[s], ...) and fires
    # as soon as its windows are staged.
    gdst = slot_of_node[col]
    c_d, ell_d = gdst // SPC, gdst % SPC
    p_d, w_d = ell_d // WPC, ell_d % WPC
    seg_of_w = np.zeros(WPC, np.int64)
    for s in range(1, NSEG):
        seg_of_w[SEGOFF[s]:] = s
    s_d = seg_of_w[w_d]
    gwin = (np.array(SEGSTARTW)[s_d]
            + c_d * np.array(SEGW)[s_d]
            + (w_d - np.array(SEGOFF)[s_d]))
    src_slot = slot_of_node[row] % SPC                 # local table row

    # ---- per-window tile counts (static, max over source cores) ----
    cnts = np.zeros((NWIN, NCORES), np.int64)
    np.add.at(cnts, (gwin, owner_src), 1)
    T = np.maximum(1, -(-cnts.max(1) // P)).astype(np.int64)   # [NWIN]
    win_tile0 = np.r_[0, np.cumsum(T)][:-1]
    ntiles = int(T.sum())
    stream = ntiles * P

    # ---- per-core gather index + dest-column streams ----
    gidx_s = np.full((NCORES, stream), SPARE_ROW, np.int64)
    dloc_s = np.full((NCORES, stream), 255, np.int64)
    for h in range(NCORES):
        sel = owner_src == h
        gw, ss, pd = gwin[sel], src_slot[sel], p_d[sel]
        o = np.argsort(gw, kind="stable")
        gw, ss, pd = gw[o], ss[o], pd[o]
        starts = np.r_[0, np.flatnonzero(np.diff(gw)) + 1]
        first = np.zeros(len(gw), np.int64)
        first[starts] = starts
        first = np.maximum.accumulate(first)
        rank = np.arange(len(gw)) - first
        pos = win_tile0[gw] * P + rank
        gidx_s[h, pos] = ss
        dloc_s[h, pos] = pd

    # ---- gather bins: consecutive whole windows, <= NTBIN tiles;
    #      forced break at segment boundaries ----
    bins = []                                          # (t_lo, nt, w_lo, nw)
    w_lo, t_lo = 0, 0
    for W in range(NWIN):
        if W > w_lo and ((win_tile0[W] + T[W] - t_lo) > NTBIN
                         or W in SEGSTARTW):
            bins.append((t_lo, int(win_tile0[W] - t_lo), w_lo, W - w_lo))
            w_lo, t_lo = W, int(win_tile0[W])
    bins.append((t_lo, ntiles - t_lo, w_lo, NWIN - w_lo))
    assert max(b[1] for b in bins) <= NTBIN

    # ---- BN folding ----
    S1c = (np.asarray(g1) / np.sqrt(np.asarray(rv1) + BN_EPS)).astype(np.float32)
    T1 = ((np.asarray(b1) - np.asarray(rm1)) * S1c + np.asarray(be1)).astype(np.float32)
    S2c = (np.asarray(g2) / np.sqrt(np.asarray(rv2) + BN_EPS)).astype(np.float32)
    T2 = ((np.asarray(b2) - np.asarray(rm2)) * S2c + np.asarray(be2)).astype(np.float32)
    W1p = (np.asarray(W1) * S1c[None, :]).astype(np.float32)
    W2p = (np.asarray(W2) * S2c[None, :]).astype(np.float32)

    # ---- per-core tensors ----
    # xT column j holds slot (j%128)*WPC + j//128 so dense tile t yields
    # u1[p, t*HID:..] = slot p*WPC + t.
    colperm = (np.arange(SPC) % P) * WPC + (np.arange(SPC) // P)
    iota = np.tile(np.arange(P, dtype=np.float32)[None, :], (P, 1))
    in_maps = []
    for c in range(NCORES):
        xs = np.zeros((SPC, IN_CH), np.float32)
        dv = np.zeros(SPC, np.float32)
        valid = node_of_slot[c] >= 0
        nd = node_of_slot[c][valid]
        xs[valid] = x[nd] * dinv[nd, None]
        dv[valid] = dinv[nd]
        xs = xs[colperm]                                # [SPC(col j), IN_CH]
        dv_im = dv.reshape(P, WPC)                      # [p, w]
        in_maps.append({
            "xT": np.ascontiguousarray(xs.T).astype(F16),
            "gidx": _wrap_idx(gidx_s[c]),
            "dloc": np.ascontiguousarray(
                dloc_s[c].reshape(ntiles, P).T).astype(F16),
            "dinv": dv_im.astype(F16),
            "iota": iota.astype(F16),
            "w1": W1p.astype(F16),
            "w2": W2p.astype(F16),
            "t1": np.tile(T1[None, :], (P, 1)).astype(F16),
            "t2": np.tile(T2[None, :], (P, 1)).astype(F16),
            "fcw": np.tile(np.asarray(fcW, np.float32).reshape(1, -1),
                           (P, 1)).astype(F16),
        })

    consts = dict(T=T.tolist(), win_tile0=win_tile0.tolist(), bins=bins,
                  ntiles=ntiles, node_of_slot=node_of_slot,
                  fcb=float(np.asarray(fcb).reshape(-1)[0]))
    return in_maps, consts


# ----------------------------------------------------------------------
# raw dma_gather (elem_size below 256B; stride multiple of 256B)
# ----------------------------------------------------------------------
def _dma_gather_raw(gp, bassmod, out_ap, in_ap, idxs_ap, num_idxs, elem_size,
                    elem_step, single_packet=True, queue_num=0):
    import concourse.mybir as mybir
    from concourse import ap_utils
    from concourse.bass import MemorySpace, exact_div, round_up_to_multiple

    assert idxs_ap.dtype == mybir.dt.int16
    assert in_ap.dtype == out_ap.dtype
    assert in_ap.space == MemorySpace.DRAM
    assert idxs_ap.space == MemorySpace.SBUF and out_ap.space == MemorySpace.SBUF
    assert ap_utils.ap_is_contiguous(out_ap.ap[1:])
    assert ap_utils.ap_is_contiguous(idxs_ap.ap[1:])
    assert in_ap.ap[-1][1] == out_ap.ap[-1][1] == elem_size
    assert out_ap.ap[0][1] * out_ap.ap[1][1] == round_up_to_multiple(num_idxs, 128)
    assert in_ap.ap[0][0] == elem_step
    stride_bytes_256 = exact_div(elem_step * mybir.dt.size(in_ap.dtype), 256)
    assert stride_bytes_256 < 256
    return gp.add_instruction(
        mybir.InstDMAGatherAnt(
            name=bassmod.get_next_instruction_name(),
            ins=[*gp.lower_ap_dma(in_ap, for_custom_bir_dma=True),
                 gp.lower_ap(idxs_ap),
                 gp.lower_val_access(gp.to_reg(num_idxs))],
            outs=[gp.lower_ap(out_ap)],
            transpose=False,
            num_idxs=num_idxs,
            elem_size=elem_size,
            stride_bytes_256=stride_bytes_256,
            gen_mode=0,
            single_packet=single_packet,
            queue_num=queue_num,
            sbuf_tokens_per_rank=0,
            sbuf_free_dim_per_rank=0,
            sbuf_free_dim_pad_per_rank=0,
            sbuf_byte_offset=0,
        ))


# ----------------------------------------------------------------------
# device program
# ----------------------------------------------------------------------
def build_bass(T, win_tile0, bins, ntiles):
    import concourse.bacc as bacc
    import concourse.bass as bassm
    import concourse.mybir as mybir
    import concourse.tile as tile
    from concourse.masks import make_identity

    f32 = mybir.dt.float32
    bf = mybir.dt.float16
    f8 = mybir.dt.float8e4
    i16 = mybir.dt.int16
    TBW8 = 256                 # fp8 table row stride in elems (256B)

    import os as _os
    nc = bacc.Bacc("TRN2", target_bir_lowering=False,
                   dynamic_dma_scratch_size=int(_os.environ.get("KSCRATCH", "49152")),
                   num_swdge_queues=1)
    xT_d = nc.dram_tensor("xT", [P, SPC], bf, kind="ExternalInput")
    gidx_d = nc.dram_tensor("gidx", [P, ntiles * 8], i16, kind="ExternalInput")
    dloc_d = nc.dram_tensor("dloc", [P, ntiles], bf, kind="ExternalInput")
    dinv_d = nc.dram_tensor("dinv", [P, WPC], bf, kind="ExternalInput")
    iota_d = nc.dram_tensor("iota", [P, P], bf, kind="ExternalInput")
    w1_d = nc.dram_tensor("w1", [IN_CH, HID], bf, kind="ExternalInput")
    w2_d = nc.dram_tensor("w2", [HID, HID2], bf, kind="ExternalInput")
    t1_d = nc.dram_tensor("t1", [P, HID], bf, kind="ExternalInput")
    t2_d = nc.dram_tensor("t2", [P, HID2], bf, kind="ExternalInput")
    fcw_d = nc.dram_tensor("fcw", [P, HID2], bf, kind="ExternalInput")
    y_d = nc.dram_tensor("y", [P, WPC], f32, kind="ExternalOutput")

    with tile.TileContext(nc) as tc:
        with (
            tc.tile_pool(name="const", bufs=1) as cpool,
            tc.tile_pool(name="work", bufs=1) as upool,
            tc.tile_pool(name="g", bufs=int(_os.environ.get("KGBUF", "3"))) as gpool,
            tc.tile_pool(name="sel", bufs=int(_os.environ.get("KSBUF", "3"))) as spool,
            tc.tile_pool(name="spre", bufs=int(_os.environ.get("KPRE", "2"))) as sprepool,
            tc.tile_pool(name="stage", bufs=2) as stpool,
            tc.tile_pool(name="zc", bufs=4) as zcpool,
            tc.tile_pool(name="tmp", bufs=1) as wpool,
            tc.tile_pool(name="pmm", bufs=2, space="PSUM") as pmm,
            tc.tile_pool(name="pagg", bufs=3, space="PSUM") as pagg,
            tc.tile_pool(name="ptr", bufs=3, space="PSUM") as ptr,
            tc.tile_pool(name="dram", bufs=1, space="DRAM") as dpool,
        ):
            # ---- DRAM scratch ----
            tab1_d = dpool.tile([SPC, TBW], bf)
            tab2_d = dpool.tile([SPC, TBW], bf)
            acc1_d = dpool.tile([NSLOT, HID], bf)
            acc2_d = dpool.tile([NSLOT, HID2], bf)
            rs1_d = dpool.tile([SPC, HID], bf)
            rs2_d = dpool.tile([SPC, HID2], bf)

            # ---- constants (w1 first: it gates the dense chain that the
            # first gather waits on; the rest is needed later) ----
            w1_t = cpool.tile([IN_CH, HID], bf)
            nc.scalar.dma_start(out=w1_t[:], in_=w1_d[:])
            dloc_t = cpool.tile([P, ntiles], bf)
            nc.scalar.dma_start(out=dloc_t[:], in_=dloc_d[:])
            dinv_t = cpool.tile([P, WPC], bf)
            nc.scalar.dma_start(out=dinv_t[:], in_=dinv_d[:])
            iota_t = cpool.tile([P, P], bf)
            nc.scalar.dma_start(out=iota_t[:], in_=iota_d[:])
            gidx_t = cpool.tile([P, ntiles * 8], i16)
            nc.scalar.dma_start(out=gidx_t[:], in_=gidx_d[:])
            ident = cpool.tile([P, P], bf)
            make_identity(nc, ident[:])

            dinv2_t = cpool.tile([P, WPC], bf)
            nc.vector.tensor_tensor(out=dinv2_t[:], in0=dinv_t[:], in1=dinv_t[:],
                                    op=mybir.AluOpType.mult)
            # f32 copy of dloc (tensor_scalar is_equal wants an f32 scalar AP)
            dlocf_t = cpool.tile([P, ntiles], f32)
            nc.vector.tensor_copy(out=dlocf_t[:], in_=dloc_t[:])

            # ---- L1 dense: u1 = (dinv*x)^T tiles @ W1p (xT in 2 chunks) ----
            u1_t = upool.tile([P, WPC * HID], bf, tag="u1")
            HWPC = WPC // 2
            for half in range(2):
                xc = stpool.tile([P, HWPC * P], bf, tag="stg64",
                                 name=f"xc{half}")
                nc.sync.dma_start(out=xc[:],
                                  in_=xT_d[:, half * HWPC * P:
                                           (half + 1) * HWPC * P])
                for b in range((HWPC + 7) // 8):
                    pm = pmm.tile([P, 512], f32, space="PSUM", tag="pm")
                    ts = range(b * 8, min((b + 1) * 8, HWPC))
                    for i, t in enumerate(ts):
                        nc.tensor.matmul(out=pm[:, i * HID:(i + 1) * HID],
                                         lhsT=xc[:, t * P:(t + 1) * P],
                                         rhs=w1_t[:], start=True, stop=True)
                    nts = len(ts)
                    t0 = half * HWPC + b * 8
                    nc.scalar.activation(
                        out=u1_t[:, t0 * HID:(t0 + nts) * HID],
                        in_=pm[:, 0:nts * HID],
                        func=mybir.ActivationFunctionType.Copy)
                    # table1 rows ell = p*WPC+w (strided 256B), sliced so the
                    # write overlaps the remaining dense groups
                    nc.sync.dma_start(
                        out=bassm.AP(tensor=tab1_d[:].tensor, offset=t0 * TBW,
                                     ap=[[WPC * TBW, P], [TBW, nts], [1, HID]]),
                        in_=u1_t[:, t0 * HID:(t0 + nts) * HID]
                            .rearrange("p (w f) -> p w f", f=HID),
                    )

            # late consts (not needed until after the L1 stream)
            w2_t = cpool.tile([HID, HID2], bf)
            nc.scalar.dma_start(out=w2_t[:], in_=w2_d[:])
            t1_t = cpool.tile([P, HID], bf)
            nc.scalar.dma_start(out=t1_t[:], in_=t1_d[:])
            t2_t = cpool.tile([P, HID2], bf)
            nc.scalar.dma_start(out=t2_t[:], in_=t2_d[:])
            fcw_t = cpool.tile([P, HID2], bf)
            nc.scalar.dma_start(out=fcw_t[:], in_=fcw_d[:])

            # ---- per-edge stream: gather + one-hot matmul aggregation ----
            def wdecode(W):
                """global window -> (segment, dest core, window-in-segment,
                agg/window column w = SEGOFF[s] + wi)"""
                s = NSEG - 1
                while W < SEGSTARTW[s]:
                    s -= 1
                rel = W - SEGSTARTW[s]
                return s, rel // SEGW[s], rel % SEGW[s]

            def build_s(bi, layer, pool=None):
                t_lo, nt = bins[bi][0], bins[bi][1]
                st = (pool or spool).tile([P, NTBIN * P], bf, tag="s",
                                          name=f"s{layer}_{t_lo}")
                for trel in range(nt):
                    nc.vector.tensor_scalar(
                        out=st[:, trel * P:(trel + 1) * P],
                        in0=iota_t[:],
                        scalar1=dlocf_t[:, t_lo + trel:t_lo + trel + 1],
                        scalar2=None, op0=mybir.AluOpType.is_equal)
                return st

            def edge_stream(tab, acc, rs, fw, nwg, layer, prebuilt=(),
                            tab_step=TBW, gdt=bf):
                """Gather dest-sorted messages, build one-hot S tiles on DVE,
                accumulate per-window sums in PSUM on the PE, stage each
                (half, core) chunk in SBUF (Act copy), write it contiguously,
                and fire the half's ReduceScatter once its last chunk is
                written (delayed by one bin so Pool desc-gen isn't stalled
                on the staging write)."""
                cur_pm = [None]
                cur_stg = [None]
                pending_rs = []

                def emit_rs(s):
                    nrows = NCORES * P * SEGW[s]
                    nc.gpsimd.collective_compute(
                        "ReduceScatter", mybir.AluOpType.add,
                        replica_groups=[list(range(NCORES))],
                        ins=[bassm.AP(tensor=acc[:].tensor,
                                      offset=SEGROW[s] * fw,
                                      ap=[[fw, nrows], [1, fw]])],
                        outs=[bassm.AP(tensor=rs[:].tensor,
                                       offset=(SEGROW[s] // NCORES) * fw,
                                       ap=[[fw, nrows // NCORES], [1, fw]])],
                    )

                for bi, (t_lo, nt, w_lo, nw) in enumerate(bins):
                    gv = gpool.tile([P, NTBIN * fw], gdt, tag=f"gv{layer}",
                                    name=f"gv{layer}_{t_lo}")
                    _dma_gather_raw(
                        nc.gpsimd, nc,
                        gv[:].rearrange("p (t f) -> p t f", f=fw)[:, 0:nt, :],
                        bassm.AP(tensor=tab[:].tensor, offset=0,
                                 ap=[[tab_step, SPC], [1, fw]]),
                        gidx_t[:, t_lo * 8:(t_lo + nt) * 8], nt * P, fw,
                        tab_step, single_packet=False, queue_num=0)
                    while pending_rs:
                        emit_rs(pending_rs.pop())
                    st = prebuilt[bi] if bi < len(prebuilt) else build_s(bi, layer)
                    for W in range(w_lo, w_lo + nw):
                        s, c_, wi = wdecode(W)
                        sw = SEGW[s]
                        g0 = (wi // nwg) * nwg
                        g1_ = min(g0 + nwg, sw)
                        if wi == 0:
                            cur_stg[0] = stpool.tile([P, sw * fw], bf,
                                                     tag="stg64",
                                                     name=f"stg{layer}_{s}_{c_}")
                        if wi == g0:
                            cur_pm[0] = pagg.tile([P, 512], f32, space="PSUM",
                                                  tag="agg",
                                                  name=f"agg{layer}_{W}")
                        wrel = wi - g0
                        for k in range(T[W]):
                            trel = win_tile0[W] - t_lo + k
                            nc.tensor.matmul(
                                out=cur_pm[0][:, wrel * fw:(wrel + 1) * fw],
                                lhsT=st[:, trel * P:(trel + 1) * P],
                                rhs=gv[:, trel * fw:(trel + 1) * fw],
                                start=(k == 0), stop=(k == T[W] - 1))
                        if wi == g1_ - 1:
                            nwv = g1_ - g0
                            nc.scalar.activation(
                                out=cur_stg[0][:, g0 * fw:g1_ * fw],
                                in_=cur_pm[0][:, 0:nwv * fw],
                                func=mybir.ActivationFunctionType.Copy)
                        if wi == sw - 1:
                            nc.sync.dma_start(
                                out=bassm.AP(tensor=acc[:].tensor,
                                             offset=(SEGROW[s]
                                                     + c_ * P * sw) * fw,
                                             ap=[[sw * fw, P],
                                                 [1, sw * fw]]),
                                in_=cur_stg[0][:])
                            if c_ == NCORES - 1:
                                pending_rs.append(s)
                while pending_rs:
                    emit_rs(pending_rs.pop())

            edge_stream(tab1_d, acc1_d, rs1_d, HID, 8, 1)

            # ---- post1 per half: z = relu(dinv*agg + dinv*u1 + T1) ----
            def precompute_self(u, dvt, tt, fw, tag):
                pre = wpool.tile([P, WPC, fw], bf, tag=f"pre{tag}")
                u3 = u[:].rearrange("p (w f) -> p w f", f=fw)
                nc.vector.tensor_tensor(
                    out=pre[:], in0=u3,
                    in1=dvt[:, :, None].to_broadcast([P, WPC, fw]),
                    op=mybir.AluOpType.mult)
                nc.vector.tensor_tensor(
                    out=pre[:], in0=pre[:],
                    in1=tt[:, None, :].to_broadcast([P, WPC, fw]),
                    op=mybir.AluOpType.add)
                return pre

            def read_agg(dst, rs, fw, s):
                # seg s's RS output rows r = p*SEGW[s]+wi -> dst cols SEGOFF+wi
                sw = SEGW[s]
                nc.sync.dma_start(
                    out=dst[:, SEGOFF[s] * fw:(SEGOFF[s] + sw) * fw],
                    in_=bassm.AP(tensor=rs[:].tensor,
                                 offset=(SEGROW[s] // NCORES) * fw,
                                 ap=[[sw * fw, P], [1, sw * fw]]),
                )

            def post(agg, pre, dvt, fw, out_t, s):
                lo, nt = SEGOFF[s], SEGW[s]
                a3 = agg[:].rearrange("p (w f) -> p w f", f=fw)
                tmp = wpool.tile([P, nt, fw], bf, tag=f"pa{fw}",
                                 name=f"pa{fw}_{s}")
                nc.vector.tensor_tensor(
                    out=tmp[:], in0=a3[:, lo:lo + nt, :],
                    in1=dvt[:, lo:lo + nt, None].to_broadcast([P, nt, fw]),
                    op=mybir.AluOpType.mult)
                nc.vector.tensor_tensor(out=tmp[:], in0=tmp[:],
                                        in1=pre[:, lo:lo + nt, :],
                                        op=mybir.AluOpType.add)
                nc.scalar.activation(
                    out=out_t[:, lo * fw:(lo + nt) * fw],
                    in_=tmp[:].rearrange("p t f -> p (t f)"),
                    func=mybir.ActivationFunctionType.Relu)

            # u1 is pre-scaled by dinv, so its self-term multiplier is dinv.
            pre1 = precompute_self(u1_t, dinv_t, t1_t, HID, "1")
            agg1 = upool.tile([P, WPC * HID], bf, tag="agg1")
            z_t = upool.tile([P, WPC * HID], bf, tag="z")
            u2_t = upool.tile([P, WPC * HID2], bf, tag="u2")
            t2v = wpool.tile([P, WPC, HID2], bf, tag="t2v")

            def dense2_seg(s):
                # u2 = z @ W2p and tab2 = dinv*u2 for seg s's window columns
                wlist = list(range(SEGOFF[s], SEGOFF[s] + SEGW[s]))
                for b in range((len(wlist) + 15) // 16):
                    pm = pmm.tile([P, 512], f32, space="PSUM", tag="pm",
                                  name=f"pm2_{s}_{b}")
                    ts = wlist[b * 16:(b + 1) * 16]
                    for s4 in range(0, len(ts), 4):
                        sub = ts[s4:s4 + 4]
                        tr = ptr.tile([HID, 512], bf, space="PSUM", tag="tr")
                        for i, t in enumerate(sub):
                            nc.tensor.transpose(
                                out=tr[:, i * P:(i + 1) * P],
                                in_=z_t[:, t * HID:(t + 1) * HID],
                                identity=ident[:])
                        zc = zcpool.tile([HID, 512], bf, tag="zc",
                                         name=f"zc{s}_{b}_{s4}")
                        nc.vector.tensor_copy(out=zc[:, 0:len(sub) * P],
                                              in_=tr[:, 0:len(sub) * P])
                        for i, t in enumerate(sub):
                            nc.tensor.matmul(
                                out=pm[:, (s4 + i) * HID2:(s4 + i + 1) * HID2],
                                lhsT=zc[:, i * P:(i + 1) * P],
                                rhs=w2_t[:], start=True, stop=True)
                    nts = len(ts)
                    b0 = ts[0]
                    nc.vector.tensor_copy(
                        out=u2_t[:, b0 * HID2:(b0 + nts) * HID2],
                        in_=pm[:, 0:nts * HID2])
                    nc.vector.tensor_tensor(
                        out=t2v[:, b0:b0 + nts, :],
                        in0=pm[:, 0:nts * HID2].rearrange("p (t f) -> p t f",
                                                          f=HID2),
                        in1=dinv_t[:, b0:b0 + nts, None]
                            .to_broadcast([P, nts, HID2]),
                        op=mybir.AluOpType.mult)
                nc.sync.dma_start(
                    out=bassm.AP(tensor=tab2_d[:].tensor,
                                 offset=SEGOFF[s] * TBW,
                                 ap=[[WPC * TBW, P], [TBW, SEGW[s]], [1, HID2]]),
                    in_=t2v[:, SEGOFF[s]:SEGOFF[s] + SEGW[s], :])

            # prebuild the first L2 S bins: pure DVE work that fills the
            # RS1-B bubble (no deps on layer-1 results)
            KPRE = int(_os.environ.get("KPRE", "2"))
            pre_s = [build_s(bi, 2, pool=sprepool) for bi in range(KPRE)]

            for s in range(NSEG):
                read_agg(agg1, rs1_d, HID, s)
                post(agg1, pre1, dinv_t, HID, z_t, s)
                dense2_seg(s)

            edge_stream(tab2_d, acc2_d, rs2_d, HID2, 16, 2, prebuilt=pre_s)

            # ---- post2 per half + fc ----
            # table2 values dinv*u2 give self term dinv2*u2; u2 is unscaled.
            pre2 = precompute_self(u2_t, dinv2_t, t2_t, HID2, "2")
            agg2 = upool.tile([P, WPC * HID2], bf, tag="agg2")
            h2_t = upool.tile([P, WPC * HID2], bf, tag="h2")
            out_t = upool.tile([P, WPC], f32, tag="out")
            for s in range(NSEG):
                read_agg(agg2, rs2_d, HID2, s)
                post(agg2, pre2, dinv_t, HID2, h2_t, s)
                o0, sw = SEGOFF[s], SEGW[s]
                prod = wpool.tile([P, sw, HID2], bf, tag="prod",
                                  name=f"prod_{s}")
                nc.vector.tensor_tensor(
                    out=prod[:],
                    in0=h2_t[:, o0 * HID2:(o0 + sw) * HID2]
                        .rearrange("p (w f) -> p w f", f=HID2),
                    in1=fcw_t[:, None, :].to_broadcast([P, sw, HID2]),
                    op=mybir.AluOpType.mult)
                nc.vector.reduce_sum(
                    out=out_t[:, o0:o0 + sw, None], in_=prod[:],
                    axis=mybir.AxisListType.X)
            nc.sync.dma_start(out=y_d[:], in_=out_t[:])

    nc.compile()
    return nc


# ----------------------------------------------------------------------
# entry points
# ----------------------------------------------------------------------
def prepare(inputs):
    inputs = {k: np.asarray(v) for k, v in inputs.items()}
    in_maps, consts = host_prep(**inputs)
    nc = build_bass(consts["T"], consts["win_tile0"], consts["bins"],
                    consts["ntiles"])
    return nc, in_maps, consts


def execute(nc, in_maps):
    from concourse.bass_utils import run_bass_kernel_spmd
    return run_bass_kernel_spmd(nc, in_maps, core_ids=list(range(NCORES)))


def unshard(res, consts):
    y = np.zeros((N_NODES, 1), np.float32)
    fcb = consts["fcb"]
    nos = consts["node_of_slot"]
    for c in range(NCORES):
        v = np.asarray(res.results[c]["y"], np.float32).reshape(-1)  # ell order
        valid = nos[c] >= 0
        y[nos[c][valid], 0] = v[valid] + fcb
    return y


def kernel(**inputs):
    nc, in_maps, consts = prepare(inputs)
    res = execute(nc, in_maps)
    return unshard(res, consts)
